# revision 1
# baseline (speedup 1.0000x reference)
"""Trainium2 Bass kernel for tied-row axial attention (MSA row attention).

Reference computation (B=1, M=128 rows, N=256 residues, D=256, H=8, DH=64):
    xn   = LayerNorm_D(x) * ln_g + ln_b
    bias = einsum('bijc,ch->bhij', edges, Wb)
    q    = (xn @ Wq).heads * DH**-0.5 ; k, v = (xn @ Wkv).heads
    qm   = q.mean(axis=m)                       (tied queries)
    dots = einsum('bihd,bmjhd->bmhij', qm, k) + bias
    attn = softmax_j(dots)                      (mask is all-ones)
    out  = (attn @ v  * sigmoid(xn @ Wg + bg)) @ Wo + bo

Distribution (8 cores, all on one device): shard MSA rows m (16/core).
The tied-query mean and the i-sharded pair bias are exchanged with
hand-rolled all-gathers: each core remote-DMA-broadcasts its 128KB bf16
shard into slot <core_id> of a staging tile on all 8 cores (~3us vs
~35us for a collective_compute, and the two transfers overlap).

v2 dataflow (all matmuls bf16):
  - LN in [n,d] layout (bn_stats) -> bf16 PE-transpose -> xn^T; vsum
    accumulated on Pool; vsum broadcast + local tree-reduce -> xnm^T.
  - edges arrive bf16; chunked DMA-transposes produce e^T tiles; bias
    matmuls run off SBUF; bias_loc broadcast; exp(bias^T) from staging.
  - S^T per (m,pair) into a 2-bank PSUM tile; exp on ACT; aw=ex*exp(b^T)
    split DVE/Pool; AV with a ones column appended to v so the softmax
    denominator falls out of the matmul; avps -> usb (DVE/ACT/Pool).
  - tails batched per 2 rows: one sums gather DMA, reciprocal, DRAM
    bounce + 2 broadcast DMAs, w (Pool) / t (Pool) multiplies, out-proj,
    one y DMA per row.
"""

import numpy as np

import concourse.bass as bass
import concourse.tile as tile
import concourse.mybir as mybir
from concourse.bass import ts
from concourse import library_config
from concourse.masks import make_identity

F32 = mybir.dt.float32
BF16 = mybir.dt.bfloat16
F8 = mybir.dt.float8e4
AF = mybir.ActivationFunctionType
ALU = mybir.AluOpType

# problem dims (hardcoded per contract)
B, M, N, D = 1, 128, 256, 256
DE = 128
H, DH = 8, 64
INNER = H * DH          # 512
NCORES = 8
M_LOC = M // NCORES     # 16 rows per core
I_LOC = N // NCORES     # 32 bias-i per core
NPAIR = H // 2          # 4 head pairs
EPS = 1e-5


def _split_multi_waits(nc, cap: int = 1):
    """This container's walrus accepts at most one sync-wait per instruction;
    spill extra Tile-emitted waits onto standalone NOPs on the same engine
    (same-engine sequential waits are semantically identical to a wait list)."""
    for f in nc.m.functions:
        for bb in f.blocks:
            out = []
            for ins in bb.instructions:
                si = ins.sync_info
                waits = list(si.on_wait) if (si is not None and si.on_wait) else []
                if len(waits) > cap:
                    spill, keep = waits[:-cap], waits[-cap:]
                    k = 0
                    while spill:
                        chunk, spill = spill[:cap], spill[cap:]
                        nop = mybir.InstNoOp(name=f"{ins.name}-sw{k}", ins=[], outs=[])
                        nop.engine = ins.engine
                        nop.sync_info = mybir.SyncInfo(on_wait=chunk, on_update=[])
                        out.append(nop)
                        k += 1
                    si.on_wait = keep
                out.append(ins)
            bb.instructions = out


def build_program(n_cores: int = NCORES, m_loc: int = M_LOC, proj_pipe: int = 8,
                  split_waits: bool = True, remote_coll: bool = False):
    """Build the SPMD Bass program (identical on every core)."""
    i_loc = N // n_cores
    n_edge_tiles = (i_loc * N) // 128     # 64 [128,128] edge tiles per core

    nc = bass.Bass()

    x_in = nc.dram_tensor("x", [m_loc, N, D], BF16, kind="ExternalInput")
    e_in = nc.dram_tensor("edges", [i_loc * N, DE], BF16, kind="ExternalInput")
    wk_in = nc.dram_tensor("wk", [D, INNER], BF16, kind="ExternalInput")
    wv_in = nc.dram_tensor("wv", [D, INNER], BF16, kind="ExternalInput")
    wg_in = nc.dram_tensor("wg", [D, INNER], BF16, kind="ExternalInput")
    wq_in = nc.dram_tensor("wq", [D, INNER], BF16, kind="ExternalInput")
    wo_in = nc.dram_tensor("wo", [INNER, D], BF16, kind="ExternalInput")
    wb_in = nc.dram_tensor("wb", [DE, H], BF16, kind="ExternalInput")
    bk_in = nc.dram_tensor("bk", [INNER], F32, kind="ExternalInput")
    bv_in = nc.dram_tensor("bv", [INNER], BF16, kind="ExternalInput")
    bg_in = nc.dram_tensor("bg", [INNER], F32, kind="ExternalInput")
    bq_in = nc.dram_tensor("bq", [INNER], F32, kind="ExternalInput")
    bo_in = nc.dram_tensor("bo", [D], F32, kind="ExternalInput")
    y_out = nc.dram_tensor("y", [m_loc, D, N], F32, kind="ExternalOutput")

    groups = [list(range(n_cores))]
    if remote_coll:
        v_rsem = nc.alloc_semaphore("vb_rsem")
        v_lsem = nc.alloc_semaphore("vb_lsem")
        v_psem = nc.alloc_semaphore("vb_psem")
        b_rsem = nc.alloc_semaphore("bb_rsem")
        b_lsem = nc.alloc_semaphore("bb_lsem")
        b_psem = nc.alloc_semaphore("bb_psem")

    with tile.TileContext(nc) as tc:
        with tc.tile_pool(name="consts", bufs=1) as consts, \
             tc.tile_pool(name="persist", bufs=1) as persist, \
             tc.tile_pool(name="psum", bufs=1, space="PSUM") as psum, \
             tc.tile_pool(name="dram", bufs=1, space="DRAM") as dram:

            if remote_coll:
                nc.gpsimd.load_library(library_config.proxy)

            # ---------------- constants / weights to SBUF ----------------
            ident = consts.tile([128, 128], BF16)
            make_identity(nc, ident)
            eps_sb = consts.tile([128, 1], F32)
            nc.vector.memset(eps_sb, EPS)
            ones_f = consts.tile([128, 1], F32)
            nc.vector.memset(ones_f, 1.0)

            # ---------------- persistent activations ----------------
            xnT = persist.tile([128, m_loc, 2, N], BF16)   # [d,(m,db),n]
            xnmT = persist.tile([128, 2, N], F8)           # allreduced sum
            qmT = persist.tile([128, NPAIR, N], BF16)      # tied queries ^T
            ebt = persist.tile([128, 2, H, N], BF16)       # exp(bias^T)
            bias_loc = persist.tile([128, 2, H, i_loc], F8)

            def load_w_dke(dram_t, name):
                t = consts.tile([128, D // 128, INNER], BF16, name=name)
                dap = dram_t[:]
                src = bass.AP(tensor=dap.tensor, offset=dap.offset,
                              ap=[[INNER, 128], [INNER * 128, D // 128], [1, INNER]])
                nc.sync.dma_start(out=t, in_=src)
                return t

            def load_bias(dram_t, nblk, name):
                t = consts.tile([128, nblk], F32, name=name)
                dap = dram_t[:]
                src = bass.AP(tensor=dap.tensor, offset=dap.offset,
                              ap=[[1, 128], [128, nblk]])
                nc.sync.dma_start(out=t, in_=src)
                return t

            XCH = 4                       # x rows per chunk DMA
            ECH = 16                      # edge tiles per chunk

            with tc.tile_pool(name="xp", bufs=2) as xp, \
                 tc.tile_pool(name="ep", bufs=1) as ep, \
                 tc.tile_pool(name="p1s", bufs=8) as p1s, \
                 tc.tile_pool(name="stg", bufs=1) as stg:

                x_ch = {}

                def load_x_chunk(c0):
                    xt = xp.tile([128, XCH, 2, D], BF16, name="x_ch")
                    xap = x_in[:]
                    src = bass.AP(
                        tensor=xap.tensor, offset=xap.offset + c0 * N * D,
                        ap=[[D, 128], [128 * D, 2 * XCH], [1, D]])
                    nc.sync.dma_start(
                        out=xt.rearrange("p a b d -> p (a b) d"), in_=src)
                    x_ch[c0] = xt

                # issue order tuned for DMA-device contention: edges
                # and x interleave up front; weights follow.
                ebf = ep.tile([128, n_edge_tiles, 128], BF16, name="ebf")
                eT = ep.tile([128, n_edge_tiles, 128], BF16, name="eT")
                eap = e_in[:]

                def load_e_chunk(ch):
                    nc.scalar.dma_start(
                        out=ebf[:, ch * ECH:(ch + 1) * ECH, :],
                        in_=bass.AP(tensor=eap.tensor,
                                    offset=eap.offset + ch * ECH * 128 * DE,
                                    ap=[[DE, 128], [128 * DE, ECH], [1, DE]]))

                def transp_e_chunk(ch):
                    nc.sync.dma_start_transpose(
                        out=eT[:, ch * ECH:(ch + 1) * ECH, :],
                        in_=ebf[:, ch * ECH:(ch + 1) * ECH, :].rearrange(
                            "p a b -> p (a b)"))

                load_x_chunk(0)
                load_e_chunk(0)
                wk_sb = load_w_dke(wk_in, "wk_sb")
                wg_sb = load_w_dke(wg_in, "wg_sb")
                load_x_chunk(XCH)
                load_e_chunk(1)
                wv_sb = load_w_dke(wv_in, "wv_sb")
                load_x_chunk(2 * XCH)
                load_e_chunk(2)
                load_x_chunk(3 * XCH)
                load_e_chunk(3)
                transp_e_chunk(0)
                transp_e_chunk(1)
                wb_sb = consts.tile([DE, H], BF16)
                nc.sync.dma_start(out=wb_sb, in_=wb_in[:])
                transp_e_chunk(2)
                transp_e_chunk(3)
                wq_sb = load_w_dke(wq_in, "wq_sb")

                wo_sb = consts.tile([128, INNER // 128, D], BF16)
                wo_ap = wo_in[:]
                nc.sync.dma_start(
                    out=wo_sb,
                    in_=bass.AP(tensor=wo_ap.tensor, offset=wo_ap.offset,
                                ap=[[D, 128], [D * 128, INNER // 128], [1, D]]))
                bk_sb = load_bias(bk_in, 4, "bk_sb")
                bg_sb = load_bias(bg_in, 4, "bg_sb")
                bq_sb = load_bias(bq_in, 4, "bq_sb")
                bo_sb = load_bias(bo_in, 2, "bo_sb")
                bvb = consts.tile([128, INNER], BF16, name="bvb")
                bv_ap = bv_in[:]
                nc.sync.dma_start(
                    out=bvb,
                    in_=bass.AP(tensor=bv_ap.tensor, offset=bv_ap.offset,
                                ap=[[0, 128], [1, INNER]]))

                # ---------------- phase 1: LN over all local rows ----------
                vsum = stg.tile([128, 2, N], F32, name="vsum")
                nc.vector.memset(vsum, 0.0)

                def ln_stats(m, nb):
                    xv = x_ch[(m // XCH) * XCH][:, m % XCH, nb, :]
                    stats = p1s.tile([128, 6], F32, name="stats")
                    nc.vector.bn_stats(out=stats, in_=xv)
                    mv = p1s.tile([128, 2], F32, name="mv")
                    nc.vector.bn_aggr(out=mv, in_=stats)
                    rstd = p1s.tile([128, 1], F32, name="rstd")
                    nc.scalar.activation(out=rstd, in_=mv[:, 1:2],
                                         func=AF.Sqrt, bias=eps_sb)
                    return xv, mv, rstd

                def ln_apply(m, nb, st):
                    xv, mv, rstd = st
                    nc.vector.reciprocal(out=rstd, in_=rstd)
                    nmu = p1s.tile([128, 1], F32, name="nmu")
                    nc.vector.scalar_tensor_tensor(
                        out=nmu, in0=mv[:, 0:1], scalar=-1.0, in1=rstd,
                        op0=ALU.mult, op1=ALU.mult)
                    xnat = p1s.tile([128, D], BF16, name="xnat")
                    nc.scalar.activation(out=xnat, in_=xv, func=AF.Identity,
                                         bias=nmu, scale=rstd)
                    tps = psum.tile([128, 2, 128], BF16, tag="mm", bufs=2,
                                    name="tps")
                    for db in range(2):
                        nc.tensor.transpose(tps[:, db, :],
                                            xnat[:, db * 128:(db + 1) * 128],
                                            ident)
                    if nb == 0:
                        nc.vector.tensor_copy(
                            out=xnT[:, m, :, nb * 128:(nb + 1) * 128], in_=tps)
                    else:
                        nc.scalar.copy(
                            out=xnT[:, m, :, nb * 128:(nb + 1) * 128], in_=tps)
                    sl = vsum[:, :, nb * 128:(nb + 1) * 128]
                    nc.vector.tensor_tensor(out=sl, in0=sl, in1=tps, op=ALU.add)

                ln_jobs = [(m, nb) for m in range(m_loc) for nb in range(2)]
                pend = None
                for job in ln_jobs:
                    st = ln_stats(*job)
                    if pend is not None:
                        ln_apply(*pend[0], pend[1])
                    pend = (job, st)
                ln_apply(*pend[0], pend[1])

                # ---------------- phase 2a: vsum exchange ------------------
                vsum_bf = stg.tile([128, 2 * N], F8, name="vsum_bf")
                nc.vector.tensor_copy(out=vsum_bf,
                                      in_=vsum.rearrange("p a n -> p (a n)"))
                xs_d = dram.tile([128 * 2 * N], F8)
                nc.sync.dma_start(out=xs_d, in_=vsum_bf)
                xr_d = dram.tile([128 * 2 * N], F8, addr_space="Shared")
                nc.gpsimd.collective_compute(
                    "AllReduce", ALU.add, replica_groups=groups,
                    ins=[xs_d[:]], outs=[xr_d[:]])
                nc.sync.dma_start(
                    out=xnmT,
                    in_=bass.AP(tensor=xr_d.tensor, offset=xr_d.offset,
                                ap=[[2 * N, 128], [N, 2], [1, N]]))

                # ---------------- phase 2b: bias matmuls + exchange --------
                for bk_i in range(n_edge_tiles // 8):
                    bps = psum.tile([128, 8, H], F32, tag="sp", bufs=2,
                                    name="bps")
                    for s in range(8):
                        t_i = bk_i * 8 + s
                        nc.tensor.matmul(out=bps[:, s, :],
                                         lhsT=eT[:, t_i, :], rhs=wb_sb,
                                         start=True, stop=True)
                    dst = bias_loc[:, :, :, bk_i * 4:(bk_i + 1) * 4]
                    nc.vector.tensor_copy(
                        out=dst,
                        in_=bps.rearrange("p (il jh) h -> p jh h il", jh=2))

                bl_d = dram.tile([128 * 2 * H * i_loc], F8)
                nc.sync.dma_start(out=bl_d, in_=bias_loc)
                bg_d = dram.tile([n_cores * 128 * 2 * H * i_loc], F8,
                                 addr_space="Shared")
                nc.gpsimd.collective_compute(
                    "AllGather", ALU.bypass, replica_groups=groups,
                    ins=[bl_d[:]], outs=[bg_d[:]])
                core_stride = 128 * 2 * H * i_loc
                ebt_stage = stg.tile([128, n_cores, 2 * H * i_loc], F8,
                                     name="ebt_stage")
                nc.sync.dma_start(
                    out=ebt_stage,
                    in_=bass.AP(tensor=bg_d.tensor, offset=bg_d.offset,
                                ap=[[2 * H * i_loc, 128],
                                    [core_stride, n_cores],
                                    [1, 2 * H * i_loc]]))
                # ebt[p, jb, h, (c, il)] = exp(stage[p, c, jb, h, il])
                for jb in range(2):
                    nc.scalar.activation(
                        out=ebt[:, jb, :, :].rearrange(
                            "p h (c i) -> p h c i", c=n_cores),
                        in_=ebt_stage.rearrange(
                            "p c (a h i) -> p a h c i", a=2, h=H)[:, jb],
                        func=AF.Exp)

            # ---------------- phase 3: projections + attention -------------
            with tc.tile_pool(name="kT", bufs=proj_pipe) as kT_pool, \
                 tc.tile_pool(name="gT", bufs=proj_pipe) as gT_pool, \
                 tc.tile_pool(name="vo", bufs=proj_pipe) as vo_pool, \
                 tc.tile_pool(name="ex", bufs=12) as ex_pool, \
                 tc.tile_pool(name="aw", bufs=4) as aw_pool, \
                 tc.tile_pool(name="usb", bufs=3) as usb_pool, \
                 tc.tile_pool(name="tl", bufs=3) as tl_pool, \
                 tc.tile_pool(name="smal", bufs=3) as smal, \
                 tc.tile_pool(name="rdram", bufs=2, space="DRAM") as rdram:

                def proj(m):
                    kT = kT_pool.tile([128, NPAIR, N], BF16, name="kT")
                    gT = gT_pool.tile([128, NPAIR, N], BF16, name="gT")
                    vo = vo_pool.tile([128, 2, H, DH + 1], BF16, name="vo")
                    for half in range(2):
                        kps = psum.tile([128, 2, N], F32, tag="mm", bufs=2,
                                        name="kps")
                        gps = psum.tile([128, 2, N], F32, tag="mm", bufs=2,
                                        name="gps")
                        for sub in range(2):
                            eb = half * 2 + sub
                            for db in range(2):
                                nc.tensor.matmul(
                                    out=kps[:, sub, :],
                                    lhsT=wk_sb[:, db, eb * 128:(eb + 1) * 128],
                                    rhs=xnT[:, m, db, :],
                                    start=(db == 0), stop=(db == 1))
                            for db in range(2):
                                nc.tensor.matmul(
                                    out=gps[:, sub, :],
                                    lhsT=wg_sb[:, db, eb * 128:(eb + 1) * 128],
                                    rhs=xnT[:, m, db, :],
                                    start=(db == 0), stop=(db == 1))
                        for sub in range(2):
                            eb = half * 2 + sub
                            nc.vector.tensor_scalar_add(
                                out=kT[:, eb, :], in0=kps[:, sub, :],
                                scalar1=bk_sb[:, eb:eb + 1])
                            gtmp = smal.tile([128, N], BF16, tag="gtmp")
                            nc.scalar.activation(
                                out=gtmp, in_=gps[:, sub, :],
                                func=AF.Tanh, bias=bg_sb[:, eb:eb + 1],
                                scale=0.5)
                            nc.gpsimd.tensor_scalar(
                                out=gT[:, eb, :], in0=gtmp, scalar1=0.5,
                                scalar2=0.5, op0=ALU.mult, op1=ALU.add)
                    for nb in range(2):
                        vps = psum.tile([128, INNER], F32, tag="mm", bufs=2,
                                        name="vps")
                        for db in range(2):
                            nc.tensor.matmul(
                                out=vps,
                                lhsT=xnT[:, m, db, nb * 128:(nb + 1) * 128],
                                rhs=wv_sb[:, db, :],
                                start=(db == 0), stop=(db == 1))
                        nc.vector.tensor_tensor(
                            out=vo[:, nb, :, 0:DH],
                            in0=vps.rearrange("p (h d) -> p h d", h=H),
                            in1=bvb.rearrange("p (h d) -> p h d", h=H),
                            op=ALU.add)
                        ones_bc = bass.AP(tensor=ones_f.tensor,
                                          offset=ones_f.offset,
                                          ap=[ones_f.ap[0], [0, H], [1, 1]])
                        nc.vector.tensor_copy(out=vo[:, nb, :, DH:DH + 1],
                                              in_=ones_bc)
                    return kT, gT, vo

                def attn(m, kT, gT, vo, usb, m2):
                    for pr in range(NPAIR):
                        sps = psum.tile([128, 2, 2, N], F32, tag="sp", bufs=2,
                                        name="sps")   # [j, eo, jb, i]
                        for eo in range(2):
                            base = eo * 64
                            for jb in range(2):
                                nc.tensor.matmul(
                                    out=sps[:, eo, jb, :],
                                    lhsT=kT[base:base + 64, pr,
                                            jb * 128:(jb + 1) * 128],
                                    rhs=qmT[base:base + 64, pr, :],
                                    start=True, stop=True)
                        ex = ex_pool.tile([128, 2, 2, N], BF16, name="ex")
                        nc.scalar.activation(out=ex, in_=sps, func=AF.Exp)
                        aw = aw_pool.tile([128, 2, 2, N], BF16, name="aw")
                        ebt_sl = ebt[:, :, 2 * pr:2 * pr + 2, :].rearrange(
                            "p jb h i -> p h jb i")
                        nc.gpsimd.tensor_tensor(out=aw, in0=ex, in1=ebt_sl,
                                                op=ALU.mult)
                        avps = psum.tile([DH + 1, 2, N], F32, tag="av",
                                         bufs=2, name="avps")
                        for eo in range(2):
                            h = 2 * pr + eo
                            for jb in range(2):
                                nc.tensor.matmul(
                                    out=avps[:, eo, :],
                                    lhsT=vo[:, jb, h, :],
                                    rhs=aw[:, eo, jb, :],
                                    start=(jb == 0), stop=(jb == 1))
                        if pr == 3:
                            nc.scalar.copy(out=usb[:, m2, pr, :, :], in_=avps)
                        else:
                            nc.vector.tensor_copy(out=usb[:, m2, pr, :, :],
                                                  in_=avps)

                def tail(ms, usb, gTs):
                    sums = smal.tile([2 * H, N], BF16, tag="sums")
                    nc.sync.dma_start(out=sums, in_=usb[64:65, :, :, :, :])
                    rm = smal.tile([2 * H, N], BF16, tag="rm")
                    with nc.allow_low_precision(reason="softmax denom bf16"):
                        nc.vector.reciprocal(out=rm, in_=sums)
                    rm_d = rdram.tile([2 * H, N], BF16, name="rm_d")
                    nc.sync.dma_start(out=rm_d, in_=rm)
                    rbc = tl_pool.tile([128, 2, NPAIR, N], BF16, name="rbc")
                    rmap = rm_d[:]
                    for eo in range(2):
                        src = bass.AP(
                            tensor=rmap.tensor,
                            offset=rmap.offset + eo * N,
                            ap=[[0, 64], [H * N, 2], [2 * N, NPAIR], [1, N]])
                        nc.sync.dma_start(out=rbc[eo * 64:(eo + 1) * 64],
                                          in_=src)
                    up = tl_pool.tile([128, 2, NPAIR, N], BF16, name="up")
                    for eo in range(2):
                        nc.sync.dma_start(out=up[eo * 64:(eo + 1) * 64],
                                          in_=usb[0:DH, :, :, eo, :])
                    w = tl_pool.tile([128, 2, NPAIR, N], BF16, name="w")
                    nc.gpsimd.tensor_tensor(out=w, in0=up, in1=rbc,
                                            op=ALU.mult)
                    t = tl_pool.tile([128, 2, NPAIR, N], BF16, name="t")
                    for m2 in range(2):
                        nc.gpsimd.tensor_tensor(out=t[:, m2], in0=w[:, m2],
                                                in1=gTs[m2], op=ALU.mult)
                    for m2, m in enumerate(ms):
                        yps = psum.tile([128, 2, N], F32, tag="mm", bufs=2,
                                        name="yps")
                        for dc in range(2):
                            for pr in range(NPAIR):
                                nc.tensor.matmul(
                                    out=yps[:, dc, :],
                                    lhsT=wo_sb[:, pr, dc * 128:(dc + 1) * 128],
                                    rhs=t[:, m2, pr, :],
                                    start=(pr == 0), stop=(pr == NPAIR - 1))
                        ysb = smal.tile([128, 2, N], F32, tag="ysb")
                        for dc in range(2):
                            nc.vector.tensor_scalar_add(
                                out=ysb[:, dc, :], in0=yps[:, dc, :],
                                scalar1=bo_sb[:, dc:dc + 1])
                        nc.sync.dma_start(
                            out=y_out[m].rearrange("(a p) n -> p a n", p=128),
                            in_=ysb)

                # tied queries: qm^T = Wq'^T @ xnm^T  (+ bq)
                for eb in range(4):
                    qps = psum.tile([128, N], F32, tag="mm", bufs=2, name="qps",
                                    padded_shape=[128, 2 * N])
                    for db in range(2):
                        nc.tensor.matmul(
                            out=qps,
                            lhsT=wq_sb[:, db, eb * 128:(eb + 1) * 128],
                            rhs=xnmT[:, db, :],
                            start=(db == 0), stop=(db == 1))
                    nc.scalar.activation(out=qmT[:, eb, :], in_=qps,
                                         func=AF.Identity,
                                         bias=bq_sb[:, eb:eb + 1])

                tiles = {}
                for m in range(min(proj_pipe, m_loc)):
                    tiles[m] = proj(m)

                usb_cur = None
                for m in range(m_loc):
                    if m % 2 == 0:
                        usb_cur = usb_pool.tile([DH + 1, 2, NPAIR, 2, N],
                                                BF16, name="usb")
                    attn(m, *tiles[m], usb_cur, m % 2)
                    nm = m + proj_pipe
                    if nm < m_loc:
                        tiles[nm] = proj(nm)
                    if m % 2 == 1:
                        tail((m - 1, m), usb_cur,
                             (tiles[m - 1][1], tiles[m][1]))
                        del tiles[m - 1], tiles[m]

    if split_waits:
        _split_multi_waits(nc)
    return nc


def prep_inputs(x, edges, ln_g, ln_b, Wq, Wkv, Wg, bg, Wo, bo, Wb,
                n_cores: int = NCORES):
    """Host-side prep: fold LayerNorm affine into the projections, shard."""
    import ml_dtypes
    bf16 = ml_dtypes.bfloat16
    scale = DH ** -0.5
    g = ln_g.astype(np.float32)
    b = ln_b.astype(np.float32)
    wk = (g[:, None] * Wkv[:, :INNER]).astype(bf16)
    wv = (g[:, None] * Wkv[:, INNER:]).astype(bf16)
    wg = (g[:, None] * Wg).astype(bf16)
    wq = (g[:, None] * Wq * (scale / M)).astype(bf16)
    bk = (b @ Wkv[:, :INNER]).astype(np.float32)
    bv = (b @ Wkv[:, INNER:]).astype(bf16)
    bgf = ((bg + b @ Wg) / 2).astype(np.float32)  # halved: tanh gate path
    bq = ((b @ Wq) * scale).astype(np.float32)

    m_loc = M // n_cores
    i_loc = N // n_cores
    shared = dict(wk=wk, wv=wv, wg=wg, wq=wq,
                  wo=np.ascontiguousarray(Wo).astype(bf16),
                  wb=np.ascontiguousarray(Wb).astype(bf16),
                  bk=bk, bv=bv, bg=bgf, bq=bq,
                  bo=np.ascontiguousarray(bo, np.float32))
    in_maps = []
    for c in range(n_cores):
        im = dict(shared)
        im["x"] = np.ascontiguousarray(x[0, c * m_loc:(c + 1) * m_loc]).astype(bf16)
        im["edges"] = np.ascontiguousarray(
            edges[0, c * i_loc:(c + 1) * i_loc].reshape(i_loc * N, DE)).astype(bf16)
        in_maps.append(im)
    return in_maps


def kernel(x, edges, mask, ln_g, ln_b, Wq, Wkv, Wg, bg, Wo, bo, Wb):
    """Full-input entry point: shard, run on 8 NeuronCores, gather."""
    del mask  # all-ones per the problem spec; softmax unmasked
    from concourse.bass_utils import run_bass_kernel_spmd

    x = np.asarray(x)
    nc = build_program(NCORES, M_LOC)
    in_maps = prep_inputs(np.asarray(x), np.asarray(edges), np.asarray(ln_g),
                          np.asarray(ln_b), np.asarray(Wq), np.asarray(Wkv),
                          np.asarray(Wg), np.asarray(bg), np.asarray(Wo),
                          np.asarray(bo), np.asarray(Wb))
    res = run_bass_kernel_spmd(nc, in_maps, list(range(NCORES)))
    outs = [res.results[c]["y"] for c in range(NCORES)]
    y = np.concatenate(outs, axis=0)          # [M, D, N]
    y = np.ascontiguousarray(np.transpose(y, (0, 2, 1)))  # [M, N, D]
    return y.reshape(B, M, N, D).astype(np.float32)



# revision 11
# speedup vs baseline: 4.6111x; 4.6111x over previous
"""Trainium2 Bass kernel for tied-row axial attention (MSA row attention).

Reference computation (B=1, M=128 rows, N=256 residues, D=256, H=8, DH=64):
    xn   = LayerNorm_D(x) * ln_g + ln_b
    bias = einsum('bijc,ch->bhij', edges, Wb)
    q    = (xn @ Wq).heads * DH**-0.5 ; k, v = (xn @ Wkv).heads
    qm   = q.mean(axis=m)                       (tied queries)
    dots = einsum('bihd,bmjhd->bmhij', qm, k) + bias
    attn = softmax_j(dots)                      (mask is all-ones)
    out  = (attn @ v  * sigmoid(xn @ Wg + bg)) @ Wo + bo

Distribution (8 cores, all on one device): shard MSA rows m (16/core).
The tied-query mean and the i-sharded pair bias are exchanged with
hand-rolled all-gathers: each core remote-DMA-broadcasts its 128KB bf16
shard into slot <core_id> of a staging tile on all 8 cores (~3us vs
~35us for a collective_compute, and the two transfers overlap).

v2 dataflow (all matmuls bf16):
  - LN in [n,d] layout (bn_stats) -> bf16 PE-transpose -> xn^T; vsum
    accumulated on Pool; vsum broadcast + local tree-reduce -> xnm^T.
  - edges arrive bf16; chunked DMA-transposes produce e^T tiles; bias
    matmuls run off SBUF; bias_loc broadcast; exp(bias^T) from staging.
  - S^T per (m,pair) into a 2-bank PSUM tile; exp on ACT; aw=ex*exp(b^T)
    split DVE/Pool; AV with a ones column appended to v so the softmax
    denominator falls out of the matmul; avps -> usb (DVE/ACT/Pool).
  - tails batched per 2 rows: one sums gather DMA, reciprocal, DRAM
    bounce + 2 broadcast DMAs, w (Pool) / t (Pool) multiplies, out-proj,
    one y DMA per row.
"""

import numpy as np

import concourse.bass as bass
import concourse.tile as tile
import concourse.mybir as mybir
from concourse.bass import ts
from concourse import library_config
from concourse.masks import make_identity

F32 = mybir.dt.float32
BF16 = mybir.dt.bfloat16
F8 = mybir.dt.float8e4
AF = mybir.ActivationFunctionType
ALU = mybir.AluOpType

# problem dims (hardcoded per contract)
B, M, N, D = 1, 128, 256, 256
DE = 128
H, DH = 8, 64
INNER = H * DH          # 512
NCORES = 8
M_LOC = M // NCORES     # 16 rows per core
I_LOC = N // NCORES     # 32 bias-i per core
NPAIR = H // 2          # 4 head pairs
EPS = 1e-5


def _split_multi_waits(nc, cap: int = 1):
    """This container's walrus accepts at most one sync-wait per instruction;
    spill extra Tile-emitted waits onto standalone NOPs on the same engine
    (same-engine sequential waits are semantically identical to a wait list)."""
    for f in nc.m.functions:
        for bb in f.blocks:
            out = []
            for ins in bb.instructions:
                si = ins.sync_info
                waits = list(si.on_wait) if (si is not None and si.on_wait) else []
                if len(waits) > cap:
                    spill, keep = waits[:-cap], waits[-cap:]
                    k = 0
                    while spill:
                        chunk, spill = spill[:cap], spill[cap:]
                        nop = mybir.InstNoOp(name=f"{ins.name}-sw{k}", ins=[], outs=[])
                        nop.engine = ins.engine
                        nop.sync_info = mybir.SyncInfo(on_wait=chunk, on_update=[])
                        out.append(nop)
                        k += 1
                    si.on_wait = keep
                out.append(ins)
            bb.instructions = out


def build_program(n_cores: int = NCORES, m_loc: int = M_LOC, proj_pipe: int = 8,
                  split_waits: bool = True, remote_coll: bool = False):
    """Build the SPMD Bass program (identical on every core)."""
    i_loc = N // n_cores
    n_edge_tiles = (i_loc * N) // 128     # 64 [128,128] edge tiles per core

    nc = bass.Bass()

    x_in = nc.dram_tensor("x", [m_loc, N, D], BF16, kind="ExternalInput")
    e_in = nc.dram_tensor("edges", [i_loc * N, DE], BF16, kind="ExternalInput")
    wk_in = nc.dram_tensor("wk", [D, INNER], BF16, kind="ExternalInput")
    wv_in = nc.dram_tensor("wv", [D, INNER], BF16, kind="ExternalInput")
    wg_in = nc.dram_tensor("wg", [D, INNER], BF16, kind="ExternalInput")
    wq_in = nc.dram_tensor("wq", [D, INNER], BF16, kind="ExternalInput")
    wo_in = nc.dram_tensor("wo", [INNER, D], BF16, kind="ExternalInput")
    wb_in = nc.dram_tensor("wb", [DE, H], BF16, kind="ExternalInput")
    bk_in = nc.dram_tensor("bk", [INNER], F32, kind="ExternalInput")
    bv_in = nc.dram_tensor("bv", [INNER], BF16, kind="ExternalInput")
    bg_in = nc.dram_tensor("bg", [INNER], F32, kind="ExternalInput")
    bq_in = nc.dram_tensor("bq", [INNER], F32, kind="ExternalInput")
    bo_in = nc.dram_tensor("bo", [D], F32, kind="ExternalInput")
    y_out = nc.dram_tensor("y", [m_loc, D, N], F32, kind="ExternalOutput")

    groups = [list(range(n_cores))]
    if remote_coll:
        v_rsem = nc.alloc_semaphore("vb_rsem")
        v_lsem = nc.alloc_semaphore("vb_lsem")
        v_psem = nc.alloc_semaphore("vb_psem")
        b_rsem = nc.alloc_semaphore("bb_rsem")
        b_lsem = nc.alloc_semaphore("bb_lsem")
        b_psem = nc.alloc_semaphore("bb_psem")

    with tile.TileContext(nc) as tc:
        with tc.tile_pool(name="consts", bufs=1) as consts, \
             tc.tile_pool(name="persist", bufs=1) as persist, \
             tc.tile_pool(name="psum", bufs=1, space="PSUM") as psum, \
             tc.tile_pool(name="dram", bufs=1, space="DRAM") as dram:

            if remote_coll:
                nc.gpsimd.load_library(library_config.proxy)

            # ---------------- constants / weights to SBUF ----------------
            ident = consts.tile([128, 128], BF16)
            make_identity(nc, ident)
            eps_sb = consts.tile([128, 1], F32)
            nc.vector.memset(eps_sb, EPS)
            ones_f = consts.tile([128, 1], F32)
            nc.vector.memset(ones_f, 1.0)

            # ---------------- persistent activations ----------------
            xnT = persist.tile([128, m_loc, 2, N], BF16)   # [d,(m,db),n]
            xnmT = persist.tile([128, 2, N], BF16)         # allreduced sum
            qmT = persist.tile([128, NPAIR, N], BF16)      # tied queries ^T
            bt = persist.tile([128, H, 2, N], BF16)        # bias^T [j,h,jb,i]
            bias_loc = persist.tile([128, 2, H, i_loc], BF16)

            def load_w_dke(dram_t, name):
                t = consts.tile([128, D // 128, INNER], BF16, name=name)
                dap = dram_t[:]
                src = bass.AP(tensor=dap.tensor, offset=dap.offset,
                              ap=[[INNER, 128], [INNER * 128, D // 128], [1, INNER]])
                nc.sync.dma_start(out=t, in_=src)
                return t

            def load_bias(dram_t, nblk, name):
                t = consts.tile([128, nblk], F32, name=name)
                dap = dram_t[:]
                src = bass.AP(tensor=dap.tensor, offset=dap.offset,
                              ap=[[1, 128], [128, nblk]])
                nc.sync.dma_start(out=t, in_=src)
                return t

            XCH = 4                       # x rows per chunk DMA
            ECH = 16                      # edge tiles per chunk

            with tc.tile_pool(name="xp", bufs=2) as xp, \
                 tc.tile_pool(name="ep", bufs=1) as ep, \
                 tc.tile_pool(name="p1s", bufs=8) as p1s, \
                 tc.tile_pool(name="stg", bufs=1) as stg:

                x_ch = {}

                def load_x_chunk(c0):
                    xt = xp.tile([128, XCH, 2, D], BF16, name="x_ch")
                    xap = x_in[:]
                    src = bass.AP(
                        tensor=xap.tensor, offset=xap.offset + c0 * N * D,
                        ap=[[D, 128], [128 * D, 2 * XCH], [1, D]])
                    nc.sync.dma_start(
                        out=xt.rearrange("p a b d -> p (a b) d"), in_=src)
                    x_ch[c0] = xt

                # issue order tuned for DMA-device contention: edges
                # and x interleave up front; weights follow.
                ebf = ep.tile([128, n_edge_tiles, 128], BF16, name="ebf")
                eT = ep.tile([128, n_edge_tiles, 128], BF16, name="eT")
                eap = e_in[:]

                def load_e_chunk(ch):
                    nc.scalar.dma_start(
                        out=ebf[:, ch * ECH:(ch + 1) * ECH, :],
                        in_=bass.AP(tensor=eap.tensor,
                                    offset=eap.offset + ch * ECH * 128 * DE,
                                    ap=[[DE, 128], [128 * DE, ECH], [1, DE]]))

                def transp_e_chunk(ch):
                    nc.sync.dma_start_transpose(
                        out=eT[:, ch * ECH:(ch + 1) * ECH, :],
                        in_=ebf[:, ch * ECH:(ch + 1) * ECH, :].rearrange(
                            "p a b -> p (a b)"))

                load_x_chunk(0)
                load_e_chunk(0)
                wk_sb = load_w_dke(wk_in, "wk_sb")
                wg_sb = load_w_dke(wg_in, "wg_sb")
                load_x_chunk(XCH)
                load_e_chunk(1)
                wv_sb = load_w_dke(wv_in, "wv_sb")
                load_x_chunk(2 * XCH)
                load_e_chunk(2)
                load_x_chunk(3 * XCH)
                load_e_chunk(3)
                transp_e_chunk(0)
                transp_e_chunk(1)
                wb_sb = consts.tile([DE, H], BF16)
                nc.sync.dma_start(out=wb_sb, in_=wb_in[:])
                transp_e_chunk(2)
                transp_e_chunk(3)
                wq_sb = load_w_dke(wq_in, "wq_sb")

                wo_sb = consts.tile([128, INNER // 128, D], BF16)
                wo_ap = wo_in[:]
                nc.sync.dma_start(
                    out=wo_sb,
                    in_=bass.AP(tensor=wo_ap.tensor, offset=wo_ap.offset,
                                ap=[[D, 128], [D * 128, INNER // 128], [1, D]]))
                bk_sb = load_bias(bk_in, 4, "bk_sb")
                bg_sb = load_bias(bg_in, 4, "bg_sb")
                bq_sb = load_bias(bq_in, 4, "bq_sb")
                bo_sb = load_bias(bo_in, 2, "bo_sb")
                bvb = consts.tile([128, INNER], BF16, name="bvb")
                bv_ap = bv_in[:]
                nc.sync.dma_start(
                    out=bvb,
                    in_=bass.AP(tensor=bv_ap.tensor, offset=bv_ap.offset,
                                ap=[[0, 128], [1, INNER]]))

                # ---------------- phase 1: LN over all local rows ----------
                vsum = stg.tile([128, 2, N], BF16, name="vsum")
                nc.gpsimd.memset(vsum, 0.0)

                def ln_stats(m, nb):
                    xv = x_ch[(m // XCH) * XCH][:, m % XCH, nb, :]
                    stats = p1s.tile([128, 6], F32, name="stats")
                    nc.vector.bn_stats(out=stats, in_=xv)
                    mv = p1s.tile([128, 2], F32, name="mv")
                    nc.vector.bn_aggr(out=mv, in_=stats)
                    rstd = p1s.tile([128, 1], F32, name="rstd")
                    nc.scalar.activation(out=rstd, in_=mv[:, 1:2],
                                         func=AF.Sqrt, bias=eps_sb)
                    return xv, mv, rstd

                def ln_apply(m, nb, st):
                    xv, mv, rstd = st
                    nc.vector.reciprocal(out=rstd, in_=rstd)
                    nmu = p1s.tile([128, 1], F32, name="nmu")
                    nc.vector.scalar_tensor_tensor(
                        out=nmu, in0=mv[:, 0:1], scalar=-1.0, in1=rstd,
                        op0=ALU.mult, op1=ALU.mult)
                    xnat = p1s.tile([128, D], BF16, name="xnat")
                    nc.scalar.activation(out=xnat, in_=xv, func=AF.Identity,
                                         bias=nmu, scale=rstd)
                    tps = psum.tile([128, 2, 128], BF16, tag="mm", bufs=2,
                                    name="tps")
                    for db in range(2):
                        nc.tensor.transpose(tps[:, db, :],
                                            xnat[:, db * 128:(db + 1) * 128],
                                            ident)
                    if nb == 0:
                        nc.vector.tensor_copy(
                            out=xnT[:, m, :, nb * 128:(nb + 1) * 128], in_=tps)
                    else:
                        nc.scalar.copy(
                            out=xnT[:, m, :, nb * 128:(nb + 1) * 128], in_=tps)
                        # row complete: fold into vsum on the (idle) Pool engine
                        nc.gpsimd.tensor_tensor(out=vsum, in0=vsum,
                                                in1=xnT[:, m, :, :], op=ALU.add)

                ln_jobs = [(m, nb) for m in range(m_loc) for nb in range(2)]
                pend = None
                for job in ln_jobs:
                    st = ln_stats(*job)
                    if pend is not None:
                        ln_apply(*pend[0], pend[1])
                    pend = (job, st)
                ln_apply(*pend[0], pend[1])

                # ---------------- phase 2: bias matmuls + fused exchange ---
                for bk_i in range(n_edge_tiles // 8):
                    bps = psum.tile([128, 8, H], F32, tag="sp", bufs=2,
                                    name="bps")
                    for s in range(8):
                        t_i = bk_i * 8 + s
                        nc.tensor.matmul(out=bps[:, s, :],
                                         lhsT=eT[:, t_i, :], rhs=wb_sb,
                                         start=True, stop=True)
                    dst = bias_loc[:, :, :, bk_i * 4:(bk_i + 1) * 4]
                    nc.vector.tensor_copy(
                        out=dst,
                        in_=bps.rearrange("p (il jh) h -> p jh h il", jh=2))

                # one AllGather carries both the q-mean partial sums and the
                # local bias^T shard: payload [vsum 128x512 | bias 128x512]
                HALF = 128 * 2 * N
                xs_d = dram.tile([2 * HALF], BF16)
                nc.sync.dma_start(
                    out=xs_d[HALF:2 * HALF],
                    in_=bias_loc.rearrange("p a h i -> p (a h i)"))
                nc.sync.dma_start(out=xs_d[0:HALF],
                                  in_=vsum.rearrange("p a n -> p (a n)"))
                bg_d = dram.tile([n_cores * 2 * HALF], BF16,
                                 addr_space="Shared")
                nc.gpsimd.collective_compute(
                    "AllGather", ALU.bypass, replica_groups=groups,
                    ins=[xs_d[:]], outs=[bg_d[:]])

            # ---------------- phase 3: projections + attention -------------
            with tc.tile_pool(name="kT", bufs=proj_pipe) as kT_pool, \
                 tc.tile_pool(name="gT", bufs=proj_pipe) as gT_pool, \
                 tc.tile_pool(name="vo", bufs=proj_pipe) as vo_pool, \
                 tc.tile_pool(name="ex", bufs=8) as ex_pool, \
                 tc.tile_pool(name="xch", bufs=1) as xch, \
                 tc.tile_pool(name="usb", bufs=3) as usb_pool, \
                 tc.tile_pool(name="tl", bufs=3) as tl_pool, \
                 tc.tile_pool(name="smal", bufs=3) as smal, \
                 tc.tile_pool(name="rdram", bufs=2, space="DRAM") as rdram:

                def proj(m):
                    kT = kT_pool.tile([128, NPAIR, N], BF16, name="kT")
                    gT = gT_pool.tile([128, NPAIR, N], BF16, name="gT")
                    vo = vo_pool.tile([128, 2, H, DH + 1], BF16, name="vo")
                    for half in range(2):
                        kps = psum.tile([128, 2, N], F32, tag="mm", bufs=2,
                                        name="kps")
                        gps = psum.tile([128, 2, N], F32, tag="mm", bufs=2,
                                        name="gps")
                        for sub in range(2):
                            eb = half * 2 + sub
                            for db in range(2):
                                nc.tensor.matmul(
                                    out=kps[:, sub, :],
                                    lhsT=wk_sb[:, db, eb * 128:(eb + 1) * 128],
                                    rhs=xnT[:, m, db, :],
                                    start=(db == 0), stop=(db == 1))
                            for db in range(2):
                                nc.tensor.matmul(
                                    out=gps[:, sub, :],
                                    lhsT=wg_sb[:, db, eb * 128:(eb + 1) * 128],
                                    rhs=xnT[:, m, db, :],
                                    start=(db == 0), stop=(db == 1))
                        for sub in range(2):
                            eb = half * 2 + sub
                            nc.vector.tensor_scalar_add(
                                out=kT[:, eb, :], in0=kps[:, sub, :],
                                scalar1=bk_sb[:, eb:eb + 1])
                            gtmp = smal.tile([128, N], BF16, tag="gtmp")
                            nc.scalar.activation(
                                out=gtmp, in_=gps[:, sub, :],
                                func=AF.Tanh, bias=bg_sb[:, eb:eb + 1],
                                scale=0.5)
                            nc.gpsimd.tensor_scalar(
                                out=gT[:, eb, :], in0=gtmp, scalar1=0.5,
                                scalar2=0.5, op0=ALU.mult, op1=ALU.add)
                    for nb in range(2):
                        vps = psum.tile([128, INNER], F32, tag="mm", bufs=2,
                                        name="vps")
                        for db in range(2):
                            nc.tensor.matmul(
                                out=vps,
                                lhsT=xnT[:, m, db, nb * 128:(nb + 1) * 128],
                                rhs=wv_sb[:, db, :],
                                start=(db == 0), stop=(db == 1))
                        nc.vector.tensor_tensor(
                            out=vo[:, nb, :, 0:DH],
                            in0=vps.rearrange("p (h d) -> p h d", h=H),
                            in1=bvb.rearrange("p (h d) -> p h d", h=H),
                            op=ALU.add)
                        ones_bc = bass.AP(tensor=ones_f.tensor,
                                          offset=ones_f.offset,
                                          ap=[ones_f.ap[0], [0, H], [1, 1]])
                        nc.vector.tensor_copy(out=vo[:, nb, :, DH:DH + 1],
                                              in_=ones_bc)
                    return kT, gT, vo

                def attn(m, kT, gT, vo, usb, m2):
                    for pr in range(NPAIR):
                        sps = psum.tile([128, 2, 2, N], F32, tag="sp", bufs=2,
                                        name="sps")   # [j, eo, jb, i]
                        for eo in range(2):
                            h = 2 * pr + eo
                            base = eo * 64
                            # seed the bank with bias^T via identity matmul,
                            # then accumulate q·k on top: exp(S+b)=exp(S)exp(b)
                            nc.tensor.matmul(
                                out=sps[:, eo, :, :].rearrange(
                                    "p a b -> p (a b)"),
                                lhsT=ident,
                                rhs=bt[:, h, :, :].rearrange("p a b -> p (a b)"),
                                start=True, stop=False)
                            for jb in range(2):
                                nc.tensor.matmul(
                                    out=sps[:, eo, jb, :],
                                    lhsT=kT[base:base + 64, pr,
                                            jb * 128:(jb + 1) * 128],
                                    rhs=qmT[base:base + 64, pr, :],
                                    start=False, stop=(jb == 1))
                        ex = ex_pool.tile([128, 2, 2, N], BF16, name="ex")
                        nc.scalar.activation(out=ex, in_=sps, func=AF.Exp)
                        avps = psum.tile([DH + 1, 2, N], F32, tag="av",
                                         bufs=2, name="avps")
                        for eo in range(2):
                            h = 2 * pr + eo
                            for jb in range(2):
                                nc.tensor.matmul(
                                    out=avps[:, eo, :],
                                    lhsT=vo[:, jb, h, :],
                                    rhs=ex[:, eo, jb, :],
                                    start=(jb == 0), stop=(jb == 1))
                        if pr == 3:
                            nc.scalar.copy(out=usb[:, m2, pr, :, :], in_=avps)
                        else:
                            nc.vector.tensor_copy(out=usb[:, m2, pr, :, :],
                                                  in_=avps)

                def tail(ms, usb, gTs):
                    sums = smal.tile([2 * H, N], BF16, tag="sums")
                    nc.sync.dma_start(out=sums, in_=usb[64:65, :, :, :, :])
                    rm = smal.tile([2 * H, N], BF16, tag="rm")
                    with nc.allow_low_precision(reason="softmax denom bf16"):
                        nc.vector.reciprocal(out=rm, in_=sums)
                    rm_d = rdram.tile([2 * H, N], BF16, name="rm_d")
                    nc.sync.dma_start(out=rm_d, in_=rm)
                    rbc = tl_pool.tile([128, 2, NPAIR, N], BF16, name="rbc")
                    rmap = rm_d[:]
                    for eo in range(2):
                        src = bass.AP(
                            tensor=rmap.tensor,
                            offset=rmap.offset + eo * N,
                            ap=[[0, 64], [H * N, 2], [2 * N, NPAIR], [1, N]])
                        nc.sync.dma_start(out=rbc[eo * 64:(eo + 1) * 64],
                                          in_=src)
                    up = tl_pool.tile([128, 2, NPAIR, N], BF16, name="up")
                    for eo in range(2):
                        nc.sync.dma_start(out=up[eo * 64:(eo + 1) * 64],
                                          in_=usb[0:DH, :, :, eo, :])
                    w = tl_pool.tile([128, 2, NPAIR, N], BF16, name="w")
                    nc.gpsimd.tensor_tensor(out=w, in0=up, in1=rbc,
                                            op=ALU.mult)
                    t = tl_pool.tile([128, 2, NPAIR, N], BF16, name="t")
                    for m2 in range(2):
                        nc.gpsimd.tensor_tensor(out=t[:, m2], in0=w[:, m2],
                                                in1=gTs[m2], op=ALU.mult)
                    for m2, m in enumerate(ms):
                        yps = psum.tile([128, 2, N], F32, tag="mm", bufs=2,
                                        name="yps")
                        for dc in range(2):
                            for pr in range(NPAIR):
                                nc.tensor.matmul(
                                    out=yps[:, dc, :],
                                    lhsT=wo_sb[:, pr, dc * 128:(dc + 1) * 128],
                                    rhs=t[:, m2, pr, :],
                                    start=(pr == 0), stop=(pr == NPAIR - 1))
                        ysb = smal.tile([128, 2, N], F32, tag="ysb")
                        for dc in range(2):
                            nc.vector.tensor_scalar_add(
                                out=ysb[:, dc, :], in0=yps[:, dc, :],
                                scalar1=bo_sb[:, dc:dc + 1])
                        nc.sync.dma_start(
                            out=y_out[m].rearrange("(a p) n -> p a n", p=128),
                            in_=ysb)

                # fill the PE pipe with projections first so the engines keep
                # working while the collective is in flight
                tiles = {}
                for m in range(min(proj_pipe, m_loc)):
                    tiles[m] = proj(m)

                # -------- collective landing: q-mean reduce + bias stage ----
                CORE = 2 * HALF
                vs_stage = xch.tile([128, n_cores, 2 * N], BF16,
                                    name="vs_stage")
                nc.sync.dma_start(
                    out=vs_stage,
                    in_=bass.AP(tensor=bg_d.tensor, offset=bg_d.offset,
                                ap=[[2 * N, 128], [CORE, n_cores], [1, 2 * N]]))
                bias_stage = xch.tile([128, n_cores, 2 * N], BF16,
                                      name="bias_stage")
                nc.sync.dma_start(
                    out=bias_stage,
                    in_=bass.AP(tensor=bg_d.tensor, offset=bg_d.offset + HALF,
                                ap=[[2 * N, 128], [CORE, n_cores], [1, 2 * N]]))
                nc.vector.tensor_tensor(out=vs_stage[:, 0:4, :],
                                        in0=vs_stage[:, 0:4, :],
                                        in1=vs_stage[:, 4:8, :], op=ALU.add)
                nc.vector.tensor_tensor(out=vs_stage[:, 0:2, :],
                                        in0=vs_stage[:, 0:2, :],
                                        in1=vs_stage[:, 2:4, :], op=ALU.add)
                nc.vector.tensor_tensor(
                    out=xnmT.rearrange("p a n -> p (a n)"),
                    in0=vs_stage[:, 0, :], in1=vs_stage[:, 1, :], op=ALU.add)
                # bt[p, h, jb, (c, il)] = stage[p, c, jb, h, il]
                for jb in range(2):
                    nc.vector.tensor_copy(
                        out=bt[:, :, jb, :].rearrange(
                            "p h (c i) -> p h c i", c=n_cores),
                        in_=bias_stage.rearrange(
                            "p c (a h i) -> p a h c i", a=2, h=H)[:, jb])

                # tied queries: qm^T = Wq'^T @ xnm^T  (+ bq)
                for eb in range(4):
                    qps = psum.tile([128, N], F32, tag="mm", bufs=2, name="qps",
                                    padded_shape=[128, 2 * N])
                    for db in range(2):
                        nc.tensor.matmul(
                            out=qps,
                            lhsT=wq_sb[:, db, eb * 128:(eb + 1) * 128],
                            rhs=xnmT[:, db, :],
                            start=(db == 0), stop=(db == 1))
                    nc.scalar.activation(out=qmT[:, eb, :], in_=qps,
                                         func=AF.Identity,
                                         bias=bq_sb[:, eb:eb + 1])

                usb_cur = None
                for m in range(m_loc):
                    if m % 2 == 0:
                        usb_cur = usb_pool.tile([DH + 1, 2, NPAIR, 2, N],
                                                BF16, name="usb")
                    attn(m, *tiles[m], usb_cur, m % 2)
                    nm = m + proj_pipe
                    if nm < m_loc:
                        tiles[nm] = proj(nm)
                    if m % 2 == 1:
                        tail((m - 1, m), usb_cur,
                             (tiles[m - 1][1], tiles[m][1]))
                        del tiles[m - 1], tiles[m]

    if split_waits:
        _split_multi_waits(nc)
    return nc


def prep_inputs(x, edges, ln_g, ln_b, Wq, Wkv, Wg, bg, Wo, bo, Wb,
                n_cores: int = NCORES):
    """Host-side prep: fold LayerNorm affine into the projections, shard."""
    import ml_dtypes
    bf16 = ml_dtypes.bfloat16
    scale = DH ** -0.5
    g = ln_g.astype(np.float32)
    b = ln_b.astype(np.float32)
    wk = (g[:, None] * Wkv[:, :INNER]).astype(bf16)
    wv = (g[:, None] * Wkv[:, INNER:]).astype(bf16)
    wg = (g[:, None] * Wg).astype(bf16)
    wq = (g[:, None] * Wq * (scale / M)).astype(bf16)
    bk = (b @ Wkv[:, :INNER]).astype(np.float32)
    bv = (b @ Wkv[:, INNER:]).astype(bf16)
    bgf = ((bg + b @ Wg) / 2).astype(np.float32)  # halved: tanh gate path
    bq = ((b @ Wq) * scale).astype(np.float32)

    m_loc = M // n_cores
    i_loc = N // n_cores
    shared = dict(wk=wk, wv=wv, wg=wg, wq=wq,
                  wo=np.ascontiguousarray(Wo).astype(bf16),
                  wb=np.ascontiguousarray(Wb).astype(bf16),
                  bk=bk, bv=bv, bg=bgf, bq=bq,
                  bo=np.ascontiguousarray(bo, np.float32))
    in_maps = []
    for c in range(n_cores):
        im = dict(shared)
        im["x"] = np.ascontiguousarray(x[0, c * m_loc:(c + 1) * m_loc]).astype(bf16)
        im["edges"] = np.ascontiguousarray(
            edges[0, c * i_loc:(c + 1) * i_loc].reshape(i_loc * N, DE)).astype(bf16)
        in_maps.append(im)
    return in_maps


def kernel(x, edges, mask, ln_g, ln_b, Wq, Wkv, Wg, bg, Wo, bo, Wb):
    """Full-input entry point: shard, run on 8 NeuronCores, gather."""
    del mask  # all-ones per the problem spec; softmax unmasked
    from concourse.bass_utils import run_bass_kernel_spmd

    x = np.asarray(x)
    nc = build_program(NCORES, M_LOC)
    in_maps = prep_inputs(np.asarray(x), np.asarray(edges), np.asarray(ln_g),
                          np.asarray(ln_b), np.asarray(Wq), np.asarray(Wkv),
                          np.asarray(Wg), np.asarray(bg), np.asarray(Wo),
                          np.asarray(bo), np.asarray(Wb))
    res = run_bass_kernel_spmd(nc, in_maps, list(range(NCORES)))
    outs = [res.results[c]["y"] for c in range(NCORES)]
    y = np.concatenate(outs, axis=0)          # [M, D, N]
    y = np.ascontiguousarray(np.transpose(y, (0, 2, 1)))  # [M, N, D]
    return y.reshape(B, M, N, D).astype(np.float32)

